# revision 7
# baseline (speedup 1.0000x reference)
"""Trainium2 Bass kernel for a char-decoder LSTM step loop.

Computation (per timestep t, PyTorch LSTM gate order i,f,g,o):
    x_t   = emb[input_t]                       (B, E)
    gates = x_t @ W_ih.T + h @ W_hh.T + b      (B, 4H)
    c     = sig(f)*c + sig(i)*tanh(g)
    h     = sig(o)*tanh(c)
    s_t   = h @ W_proj.T + b_proj              (B, V)
Returns (scores(L,B,V), (h(1,B,H), c(1,B,H))).

Strategy: data-parallel over batch across 8 NeuronCores (B=16384 -> 2048/core).
On-chip layout keeps the recurrent state TRANSPOSED (H on partitions, batch on
the free dim) so the W_hh matmul needs no per-step transposes:
    gates.T (4H, B) = W_hh.T(stationary).T @ h.T(moving) + W_ih.T.T @ x.T
Gate tiles are produced as 8 chunks of (128 gate-rows, B) in PSUM, activated on
ScalarE (bias fused, sigmoid/tanh share one table set), c-state kept fp32 on
VectorE, h cast to bf16 for the next matmul.  The per-step projection runs
batch-major (lhsT = slice of transposed h state) so scores land in PSUM already
in output layout.  The embedding gather is done on the host (numpy fancy
indexing), as is the final h/c transpose; both are cheap weight/layout-only
transforms.
"""

import os
import sys

import numpy as np

sys.path.insert(0, "/opt/trn_rl_repo")

import ml_dtypes

bf16 = ml_dtypes.bfloat16

L, B, H, E, V = 32, 16384, 256, 50, 96
N_CORES = 8
BL = B // N_CORES  # per-core batch

_F32 = None
_BF16 = None


def _dt():
    import concourse.mybir as mybir

    return mybir.dt.float32, mybir.dt.bfloat16


def build_nc(bl: int, steps: int):
    """Build the Bass program for one core processing `bl` batch elements for
    `steps` timesteps. Returns the compiled Bass object."""
    import concourse.mybir as mybir
    from concourse import bacc
    import concourse.tile as tile

    f32 = mybir.dt.float32
    b16 = mybir.dt.bfloat16
    ACT = mybir.ActivationFunctionType

    assert bl % 128 == 0
    KT = H // 128  # 2 k-tiles over the hidden dim
    MT = 4 * H // 128  # 8 gate-row chunks

    nc = bacc.Bacc("TRN2", target_bir_lowering=False, debug=False)

    xT_d = nc.dram_tensor("xT", [steps, E, bl], b16, kind="ExternalInput")
    h0T_d = nc.dram_tensor("h0T", [H, bl], f32, kind="ExternalInput")
    c0T_d = nc.dram_tensor("c0T", [H, bl], f32, kind="ExternalInput")
    whhT_d = nc.dram_tensor("whhT", [H, 4 * H], b16, kind="ExternalInput")
    wihT_d = nc.dram_tensor("wihT", [E, 4 * H], b16, kind="ExternalInput")
    wprojT_d = nc.dram_tensor("wprojT", [H, V], b16, kind="ExternalInput")
    bias_d = nc.dram_tensor("bias", [128, MT], f32, kind="ExternalInput")
    bproj_d = nc.dram_tensor("bprojr", [128, V], f32, kind="ExternalInput")

    scores_d = nc.dram_tensor("scores", [steps, bl, V], f32, kind="ExternalOutput")
    houtT_d = nc.dram_tensor("houtT", [H, bl], f32, kind="ExternalOutput")
    coutT_d = nc.dram_tensor("coutT", [H, bl], f32, kind="ExternalOutput")

    # chunking of the per-core batch (free dim)
    NH = 1024 if bl % 1024 == 0 else bl  # ACT/DVE chunk
    NN = 512 if NH % 512 == 0 else NH  # matmul free-dim chunk

    with tile.TileContext(nc) as tc:
        with (
            tc.tile_pool(name="singles", bufs=1) as singles,
            tc.tile_pool(name="gates", bufs=2) as gates_pool,
            tc.tile_pool(name="sc", bufs=2) as sc_pool,
            tc.tile_pool(name="gpsum", bufs=3, space="PSUM") as gpsum,
            tc.tile_pool(name="ppsum", bufs=2, space="PSUM") as ppsum,
        ):
            # --- constants / weights ---
            whh_sb = singles.tile([128, KT, 4 * H], b16)
            nc.sync.dma_start(
                whh_sb[:], whhT_d.ap().rearrange("(kt p) m -> p kt m", p=128)
            )
            # W_ih.T twice: at partitions 0..E-1 and 64..64+E-1 so pairs of
            # x-matmuls can run concurrently in disjoint PE row groups.
            wih_sb = singles.tile([128, 4 * H], b16)
            nc.vector.memset(wih_sb[:], 0.0)
            nc.sync.dma_start(wih_sb[:E, :], wihT_d.ap())
            nc.sync.dma_start(wih_sb[64 : 64 + E, :], wihT_d.ap())
            wproj_sb = singles.tile([128, KT, V], b16)
            nc.sync.dma_start(
                wproj_sb[:], wprojT_d.ap().rearrange("(kt p) v -> p kt v", p=128)
            )
            bias_sb = singles.tile([128, MT], f32)
            nc.sync.dma_start(bias_sb[:], bias_d.ap())
            bproj_sb = singles.tile([128, V], f32)
            nc.sync.dma_start(bproj_sb[:], bproj_d.ap())

            # --- state ---
            hbf = singles.tile([128, KT, bl], b16)  # h in bf16 (matmul operand)
            cT = singles.tile([128, KT, bl], f32)  # c in fp32
            h32 = singles.tile([128, KT, bl], f32)  # fp32 h (final output only)
            nc.sync.dma_start(h32[:], h0T_d.ap().rearrange("(kt p) n -> p kt n", p=128))
            nc.vector.tensor_copy(hbf[:], h32[:])
            nc.sync.dma_start(cT[:], c0T_d.ap().rearrange("(kt p) n -> p kt n", p=128))

            # x double buffers, duplicated at partition 64 for row packing
            xts = []
            for i in range(2):
                t_ = singles.tile([128, bl], b16, tag=f"xt{i}")
                xts.append(t_)
            nc.sync.dma_start(xts[0][:E, :], xT_d.ap()[0])
            nc.sync.dma_start(xts[0][64 : 64 + E, :], xT_d.ap()[0])

            n_nh = bl // NH
            n_nn = NH // NN
            n_mb = NH // 128

            for t in range(steps):
                xt = xts[t % 2]
                if t + 1 < steps:
                    nc.sync.dma_start(xts[(t + 1) % 2][:E, :], xT_d.ap()[t + 1])
                    nc.sync.dma_start(
                        xts[(t + 1) % 2][64 : 64 + E, :], xT_d.ap()[t + 1]
                    )

                score_sb = sc_pool.tile([128, bl // 128, V], f32, tag="score")

                for nh in range(n_nh):
                    h0_, h1_ = nh * NH, (nh + 1) * NH
                    nsl = slice(h0_, h1_)

                    # per-chunk gate tiles (i,f,g fp32 feed the c update;
                    # o and tanh_c bf16 feed only the h/scores path)
                    sig_i = gates_pool.tile([128, KT, NH], f32, tag="sig_i")
                    sig_f = gates_pool.tile([128, KT, NH], f32, tag="sig_f")
                    tg = gates_pool.tile([128, KT, NH], f32, tag="tg")
                    sig_o = gates_pool.tile([128, KT, NH], b16, tag="sig_o")
                    thc = gates_pool.tile([128, KT, NH], b16, tag="thc")
                    gdst = [sig_i, sig_i, sig_f, sig_f, tg, tg, sig_o, sig_o]
                    gfun = [ACT.Sigmoid] * 4 + [ACT.Tanh] * 2 + [ACT.Sigmoid] * 2

                    for mp in range(MT // 2):
                        m0, m1 = 2 * mp, 2 * mp + 1
                        ps0 = gpsum.tile([128, NH], f32, tag="gps")
                        ps1 = gpsum.tile([128, NH], f32, tag="gps")
                        for nn_ in range(n_nn):
                            a = nh * NH + nn_ * NN
                            sl = slice(a, a + NN)
                            pl0 = ps0[:, nn_ * NN : (nn_ + 1) * NN]
                            pl1 = ps1[:, nn_ * NN : (nn_ + 1) * NN]
                            for k in range(KT):
                                nc.tensor.matmul(
                                    pl0,
                                    whh_sb[:, k, m0 * 128 : (m0 + 1) * 128],
                                    hbf[:, k, sl],
                                    start=(k == 0),
                                    stop=False,
                                )
                                nc.tensor.matmul(
                                    pl1,
                                    whh_sb[:, k, m1 * 128 : (m1 + 1) * 128],
                                    hbf[:, k, sl],
                                    start=(k == 0),
                                    stop=False,
                                )
                            # x pair: concurrent in PE row groups 0-63 / 64-127
                            nc.tensor.matmul(
                                pl0,
                                wih_sb[:E, m0 * 128 : (m0 + 1) * 128],
                                xt[:E, sl],
                                start=False,
                                stop=True,
                                tile_position=(0, 0),
                            )
                            nc.tensor.matmul(
                                pl1,
                                wih_sb[64 : 64 + E, m1 * 128 : (m1 + 1) * 128],
                                xt[64 : 64 + E, sl],
                                start=False,
                                stop=True,
                                tile_position=(64, 0),
                            )
                        nc.scalar.activation(
                            out=gdst[m0][:, m0 % 2, :],
                            in_=ps0[:],
                            func=gfun[m0],
                            bias=bias_sb[:, m0 : m0 + 1],
                        )
                        nc.scalar.activation(
                            out=gdst[m1][:, m1 % 2, :],
                            in_=ps1[:],
                            func=gfun[m1],
                            bias=bias_sb[:, m1 : m1 + 1],
                        )

                    # c = sig_f*c + sig_i*tanh(g); h = sig_o*tanh(c)
                    nc.gpsimd.tensor_mul(tg[:], tg[:], sig_i[:])
                    nc.vector.tensor_mul(cT[:, :, nsl], cT[:, :, nsl], sig_f[:])
                    nc.vector.tensor_add(cT[:, :, nsl], cT[:, :, nsl], tg[:])
                    nc.scalar.activation(
                        out=thc[:], in_=cT[:, :, nsl], func=ACT.Tanh
                    )
                    nc.vector.tensor_mul(hbf[:, :, nsl], sig_o[:], thc[:])
                    if t == steps - 1:
                        nc.vector.tensor_mul(h32[:, :, nsl], sig_o[:], thc[:])

                    # projection, 4 batch-chunks per PSUM bank
                    for g0 in range(0, n_mb, 4):
                        gs = min(4, n_mb - g0)
                        pp = ppsum.tile([128, 4, V], f32, tag="pps")
                        for j in range(gs):
                            col = nh * NH + (g0 + j) * 128
                            nc.tensor.matmul(
                                pp[:, j, :],
                                hbf[:, 0, col : col + 128],
                                wproj_sb[:, 0, :],
                                start=True,
                                stop=False,
                            )
                            nc.tensor.matmul(
                                pp[:, j, :],
                                hbf[:, 1, col : col + 128],
                                wproj_sb[:, 1, :],
                                start=False,
                                stop=True,
                            )
                        gcol = nh * n_mb + g0
                        nc.vector.tensor_add(
                            score_sb[:, gcol : gcol + gs, :],
                            pp[:, :gs, :],
                            bproj_sb[:, None, :].to_broadcast((128, gs, V)),
                        )

                nc.sync.dma_start(
                    scores_d.ap()[t].rearrange("(mb p) v -> p mb v", p=128),
                    score_sb[:],
                )

            nc.sync.dma_start(
                houtT_d.ap().rearrange("(kt p) n -> p kt n", p=128), h32[:]
            )
            nc.sync.dma_start(
                coutT_d.ap().rearrange("(kt p) n -> p kt n", p=128), cT[:]
            )

    nc.compile()
    return nc


def _prep_host(input, h0, c0, emb, W_ih, W_hh, b_ih, b_hh, W_proj, b_proj, bl, steps):
    """Host-side data prep. Returns per-core in_maps."""
    input = np.asarray(input)
    emb = np.asarray(emb, dtype=np.float32)
    x = emb[input]  # (L, B, E) f32
    xT = np.ascontiguousarray(x.transpose(0, 2, 1)).astype(bf16)  # (L, E, B)
    h0T = np.ascontiguousarray(np.asarray(h0, np.float32)[0].T)  # (H, B)
    c0T = np.ascontiguousarray(np.asarray(c0, np.float32)[0].T)
    whhT = np.ascontiguousarray(np.asarray(W_hh, np.float32).T).astype(bf16)
    wihT = np.ascontiguousarray(np.asarray(W_ih, np.float32).T).astype(bf16)
    wprojT = np.ascontiguousarray(np.asarray(W_proj, np.float32).T).astype(bf16)
    b = (np.asarray(b_ih, np.float32) + np.asarray(b_hh, np.float32)).astype(
        np.float32
    )
    bias = np.ascontiguousarray(b.reshape(4 * H // 128, 128).T)  # (128, MT)
    bprojr = np.ascontiguousarray(
        np.tile(np.asarray(b_proj, np.float32)[None, :], (128, 1))
    )

    n_cores = x.shape[1] // bl
    in_maps = []
    for c in range(n_cores):
        sl = slice(c * bl, (c + 1) * bl)
        in_maps.append(
            {
                "xT": np.ascontiguousarray(xT[:steps, :, sl]),
                "h0T": np.ascontiguousarray(h0T[:, sl]),
                "c0T": np.ascontiguousarray(c0T[:, sl]),
                "whhT": whhT,
                "wihT": wihT,
                "wprojT": wprojT,
                "bias": bias,
                "bprojr": bprojr,
            }
        )
    return in_maps


_NC_CACHE = {}
LAST_RESULTS = None  # BassKernelResults of the most recent run (for profiling)


def _get_nc(bl, steps):
    key = (bl, steps)
    if key not in _NC_CACHE:
        _NC_CACHE[key] = build_nc(bl, steps)
    return _NC_CACHE[key]


def kernel(input, h0, c0, emb, W_ih, W_hh, b_ih, b_hh, W_proj, b_proj):
    from concourse.bass_utils import run_bass_kernel_spmd

    nc = _get_nc(BL, L)
    in_maps = _prep_host(
        input, h0, c0, emb, W_ih, W_hh, b_ih, b_hh, W_proj, b_proj, BL, L
    )
    try:
        out = run_bass_kernel_spmd(nc, in_maps, core_ids=list(range(N_CORES)))
    except Exception:
        if os.environ.pop("BASS_TRACE", None):  # retry without profiling
            out = run_bass_kernel_spmd(nc, in_maps, core_ids=list(range(N_CORES)))
        else:
            raise
    global LAST_RESULTS
    LAST_RESULTS = out
    res = out.results

    scores = np.empty((L, B, V), np.float32)
    h = np.empty((1, B, H), np.float32)
    c = np.empty((1, B, H), np.float32)
    for ci in range(N_CORES):
        sl = slice(ci * BL, (ci + 1) * BL)
        scores[:, sl, :] = res[ci]["scores"]
        h[0, sl, :] = res[ci]["houtT"].T
        c[0, sl, :] = res[ci]["coutT"].T
    return scores, (h, c)


# revision 8
# speedup vs baseline: 1.1583x; 1.1583x over previous
"""Trainium2 Bass kernel for a char-decoder LSTM step loop.

Computation (per timestep t, PyTorch LSTM gate order i,f,g,o):
    x_t   = emb[input_t]                       (B, E)
    gates = x_t @ W_ih.T + h @ W_hh.T + b      (B, 4H)
    c     = sig(f)*c + sig(i)*tanh(g)
    h     = sig(o)*tanh(c)
    s_t   = h @ W_proj.T + b_proj              (B, V)
Returns (scores(L,B,V), (h(1,B,H), c(1,B,H))).

Strategy: data-parallel over batch across 8 NeuronCores (B=16384 -> 2048/core).
On-chip layout keeps the recurrent state TRANSPOSED (H on partitions, batch on
the free dim) so the W_hh matmul needs no per-step transposes:
    gates.T (4H, B) = W_hh.T(stationary).T @ h.T(moving) + W_ih.T.T @ x.T
Gate tiles are produced as 8 chunks of (128 gate-rows, B) in PSUM, activated on
ScalarE (bias fused, sigmoid/tanh share one table set), c-state kept fp32 on
VectorE, h cast to bf16 for the next matmul.  The per-step projection runs
batch-major (lhsT = slice of transposed h state) so scores land in PSUM already
in output layout.  The embedding gather is done on the host (numpy fancy
indexing), as is the final h/c transpose; both are cheap weight/layout-only
transforms.
"""

import os
import sys

import numpy as np

sys.path.insert(0, "/opt/trn_rl_repo")

import ml_dtypes

bf16 = ml_dtypes.bfloat16

L, B, H, E, V = 32, 16384, 256, 50, 96
N_CORES = 8
BL = B // N_CORES  # per-core batch

_F32 = None
_BF16 = None


def _dt():
    import concourse.mybir as mybir

    return mybir.dt.float32, mybir.dt.bfloat16


def build_nc(bl: int, steps: int):
    """Build the Bass program for one core processing `bl` batch elements for
    `steps` timesteps. Returns the compiled Bass object."""
    import concourse.mybir as mybir
    from concourse import bacc
    import concourse.tile as tile

    f32 = mybir.dt.float32
    b16 = mybir.dt.bfloat16
    ACT = mybir.ActivationFunctionType

    assert bl % 128 == 0
    KT = H // 128  # 2 k-tiles over the hidden dim
    MT = 4 * H // 128  # 8 gate-row chunks

    nc = bacc.Bacc("TRN2", target_bir_lowering=False, debug=False)

    xT_d = nc.dram_tensor("xT", [steps, E, bl], b16, kind="ExternalInput")
    h0T_d = nc.dram_tensor("h0T", [H, bl], f32, kind="ExternalInput")
    c0T_d = nc.dram_tensor("c0T", [H, bl], f32, kind="ExternalInput")
    whhT_d = nc.dram_tensor("whhT", [H, 4 * H], b16, kind="ExternalInput")
    wihT_d = nc.dram_tensor("wihT", [E, 4 * H], b16, kind="ExternalInput")
    wprojT_d = nc.dram_tensor("wprojT", [H, V], b16, kind="ExternalInput")
    bias_d = nc.dram_tensor("bias", [128, MT], f32, kind="ExternalInput")
    bproj_d = nc.dram_tensor("bprojr", [128, V], f32, kind="ExternalInput")

    scores_d = nc.dram_tensor("scores", [steps, bl, V], f32, kind="ExternalOutput")
    houtT_d = nc.dram_tensor("houtT", [H, bl], f32, kind="ExternalOutput")
    coutT_d = nc.dram_tensor("coutT", [H, bl], f32, kind="ExternalOutput")

    # chunking of the per-core batch (free dim)
    NH = 1024 if bl % 1024 == 0 else bl  # ACT/DVE chunk
    NN = 512 if NH % 512 == 0 else NH  # matmul free-dim chunk

    with tile.TileContext(nc) as tc:
        with (
            tc.tile_pool(name="singles", bufs=1) as singles,
            tc.tile_pool(name="gates", bufs=2) as gates_pool,
            tc.tile_pool(name="sc", bufs=2) as sc_pool,
            tc.tile_pool(name="gpsum", bufs=3, space="PSUM") as gpsum,
            tc.tile_pool(name="ppsum", bufs=2, space="PSUM") as ppsum,
        ):
            # --- constants / weights ---
            whh_sb = singles.tile([128, KT, 4 * H], b16)
            nc.sync.dma_start(
                whh_sb[:], whhT_d.ap().rearrange("(kt p) m -> p kt m", p=128)
            )
            # W_ih.T twice: at partitions 0..E-1 and 64..64+E-1 so pairs of
            # x-matmuls can run concurrently in disjoint PE row groups.
            wih_sb = singles.tile([128, 4 * H], b16)
            nc.vector.memset(wih_sb[:], 0.0)
            nc.sync.dma_start(wih_sb[:E, :], wihT_d.ap())
            nc.sync.dma_start(wih_sb[64 : 64 + E, :], wihT_d.ap())
            wproj_sb = singles.tile([128, KT, V], b16)
            nc.sync.dma_start(
                wproj_sb[:], wprojT_d.ap().rearrange("(kt p) v -> p kt v", p=128)
            )
            bias_sb = singles.tile([128, MT], f32)
            nc.sync.dma_start(bias_sb[:], bias_d.ap())
            bproj_sb = singles.tile([128, V], f32)
            nc.sync.dma_start(bproj_sb[:], bproj_d.ap())

            # --- state ---
            hbf = singles.tile([128, KT, bl], b16)  # h in bf16 (matmul operand)
            cT = singles.tile([128, KT, bl], f32)  # c in fp32
            h32 = singles.tile([128, KT, bl], f32)  # fp32 h (final output only)
            nc.sync.dma_start(h32[:], h0T_d.ap().rearrange("(kt p) n -> p kt n", p=128))
            nc.vector.tensor_copy(hbf[:], h32[:])
            nc.sync.dma_start(cT[:], c0T_d.ap().rearrange("(kt p) n -> p kt n", p=128))

            # x double buffers, duplicated at partition 64 for row packing
            xts = []
            for i in range(2):
                t_ = singles.tile([128, bl], b16, tag=f"xt{i}")
                xts.append(t_)
            nc.sync.dma_start(xts[0][:E, :], xT_d.ap()[0])
            nc.sync.dma_start(xts[0][64 : 64 + E, :], xT_d.ap()[0])

            n_nh = bl // NH
            n_nn = NH // NN
            n_mb = NH // 128

            for t in range(steps):
                xt = xts[t % 2]
                if t + 1 < steps:
                    nc.sync.dma_start(xts[(t + 1) % 2][:E, :], xT_d.ap()[t + 1])
                    nc.sync.dma_start(
                        xts[(t + 1) % 2][64 : 64 + E, :], xT_d.ap()[t + 1]
                    )

                score_sb = sc_pool.tile([128, bl // 128, V], f32, tag="score")

                for nh in range(n_nh):
                    h0_, h1_ = nh * NH, (nh + 1) * NH
                    nsl = slice(h0_, h1_)

                    # per-chunk gate tiles (i,f,g fp32 feed the c update;
                    # o and tanh_c bf16 feed only the h/scores path)
                    sig_i = gates_pool.tile([128, KT, NH], f32, tag="sig_i")
                    sig_f = gates_pool.tile([128, KT, NH], f32, tag="sig_f")
                    tg = gates_pool.tile([128, KT, NH], f32, tag="tg")
                    sig_o = gates_pool.tile([128, KT, NH], b16, tag="sig_o")
                    thc = gates_pool.tile([128, KT, NH], b16, tag="thc")
                    gdst = [sig_i, sig_i, sig_f, sig_f, tg, tg, sig_o, sig_o]
                    gfun = [ACT.Sigmoid] * 4 + [ACT.Tanh] * 2 + [ACT.Sigmoid] * 2

                    for mp in range(MT // 2):
                        m0, m1 = 2 * mp, 2 * mp + 1
                        ps0 = gpsum.tile([128, NH], f32, tag="gps")
                        ps1 = gpsum.tile([128, NH], f32, tag="gps")
                        for nn_ in range(n_nn):
                            a = nh * NH + nn_ * NN
                            sl = slice(a, a + NN)
                            pl0 = ps0[:, nn_ * NN : (nn_ + 1) * NN]
                            pl1 = ps1[:, nn_ * NN : (nn_ + 1) * NN]
                            for k in range(KT):
                                nc.tensor.matmul(
                                    pl0,
                                    whh_sb[:, k, m0 * 128 : (m0 + 1) * 128],
                                    hbf[:, k, sl],
                                    start=(k == 0),
                                    stop=False,
                                )
                                nc.tensor.matmul(
                                    pl1,
                                    whh_sb[:, k, m1 * 128 : (m1 + 1) * 128],
                                    hbf[:, k, sl],
                                    start=(k == 0),
                                    stop=False,
                                )
                            # x pair: concurrent in PE row groups 0-63 / 64-127
                            nc.tensor.matmul(
                                pl0,
                                wih_sb[:E, m0 * 128 : (m0 + 1) * 128],
                                xt[:E, sl],
                                start=False,
                                stop=True,
                                tile_position=(0, 0),
                            )
                            nc.tensor.matmul(
                                pl1,
                                wih_sb[64 : 64 + E, m1 * 128 : (m1 + 1) * 128],
                                xt[64 : 64 + E, sl],
                                start=False,
                                stop=True,
                                tile_position=(64, 0),
                            )
                        nc.scalar.activation(
                            out=gdst[m0][:, m0 % 2, :],
                            in_=ps0[:],
                            func=gfun[m0],
                            bias=bias_sb[:, m0 : m0 + 1],
                        )
                        nc.scalar.activation(
                            out=gdst[m1][:, m1 % 2, :],
                            in_=ps1[:],
                            func=gfun[m1],
                            bias=bias_sb[:, m1 : m1 + 1],
                        )

                    # c = sig_f*c + sig_i*tanh(g); h = sig_o*tanh(c)
                    nc.vector.tensor_mul(tg[:], tg[:], sig_i[:])
                    nc.vector.tensor_mul(cT[:, :, nsl], cT[:, :, nsl], sig_f[:])
                    nc.vector.tensor_add(cT[:, :, nsl], cT[:, :, nsl], tg[:])
                    nc.scalar.activation(
                        out=thc[:], in_=cT[:, :, nsl], func=ACT.Tanh
                    )
                    nc.vector.tensor_mul(hbf[:, :, nsl], sig_o[:], thc[:])
                    if t == steps - 1:
                        nc.vector.tensor_mul(h32[:, :, nsl], sig_o[:], thc[:])

                    # projection, 4 batch-chunks per PSUM bank
                    for g0 in range(0, n_mb, 4):
                        gs = min(4, n_mb - g0)
                        pp = ppsum.tile([128, 4, V], f32, tag="pps")
                        for j in range(gs):
                            col = nh * NH + (g0 + j) * 128
                            nc.tensor.matmul(
                                pp[:, j, :],
                                hbf[:, 0, col : col + 128],
                                wproj_sb[:, 0, :],
                                start=True,
                                stop=False,
                            )
                            nc.tensor.matmul(
                                pp[:, j, :],
                                hbf[:, 1, col : col + 128],
                                wproj_sb[:, 1, :],
                                start=False,
                                stop=True,
                            )
                        gcol = nh * n_mb + g0
                        nc.vector.tensor_add(
                            score_sb[:, gcol : gcol + gs, :],
                            pp[:, :gs, :],
                            bproj_sb[:, None, :].to_broadcast((128, gs, V)),
                        )

                nc.sync.dma_start(
                    scores_d.ap()[t].rearrange("(mb p) v -> p mb v", p=128),
                    score_sb[:],
                )

            nc.sync.dma_start(
                houtT_d.ap().rearrange("(kt p) n -> p kt n", p=128), h32[:]
            )
            nc.sync.dma_start(
                coutT_d.ap().rearrange("(kt p) n -> p kt n", p=128), cT[:]
            )

    nc.compile()
    return nc


def _prep_host(input, h0, c0, emb, W_ih, W_hh, b_ih, b_hh, W_proj, b_proj, bl, steps):
    """Host-side data prep. Returns per-core in_maps."""
    input = np.asarray(input)
    emb = np.asarray(emb, dtype=np.float32)
    x = emb[input]  # (L, B, E) f32
    xT = np.ascontiguousarray(x.transpose(0, 2, 1)).astype(bf16)  # (L, E, B)
    h0T = np.ascontiguousarray(np.asarray(h0, np.float32)[0].T)  # (H, B)
    c0T = np.ascontiguousarray(np.asarray(c0, np.float32)[0].T)
    whhT = np.ascontiguousarray(np.asarray(W_hh, np.float32).T).astype(bf16)
    wihT = np.ascontiguousarray(np.asarray(W_ih, np.float32).T).astype(bf16)
    wprojT = np.ascontiguousarray(np.asarray(W_proj, np.float32).T).astype(bf16)
    b = (np.asarray(b_ih, np.float32) + np.asarray(b_hh, np.float32)).astype(
        np.float32
    )
    bias = np.ascontiguousarray(b.reshape(4 * H // 128, 128).T)  # (128, MT)
    bprojr = np.ascontiguousarray(
        np.tile(np.asarray(b_proj, np.float32)[None, :], (128, 1))
    )

    n_cores = x.shape[1] // bl
    in_maps = []
    for c in range(n_cores):
        sl = slice(c * bl, (c + 1) * bl)
        in_maps.append(
            {
                "xT": np.ascontiguousarray(xT[:steps, :, sl]),
                "h0T": np.ascontiguousarray(h0T[:, sl]),
                "c0T": np.ascontiguousarray(c0T[:, sl]),
                "whhT": whhT,
                "wihT": wihT,
                "wprojT": wprojT,
                "bias": bias,
                "bprojr": bprojr,
            }
        )
    return in_maps


_NC_CACHE = {}
LAST_RESULTS = None  # BassKernelResults of the most recent run (for profiling)


def _get_nc(bl, steps):
    key = (bl, steps)
    if key not in _NC_CACHE:
        _NC_CACHE[key] = build_nc(bl, steps)
    return _NC_CACHE[key]


def kernel(input, h0, c0, emb, W_ih, W_hh, b_ih, b_hh, W_proj, b_proj):
    from concourse.bass_utils import run_bass_kernel_spmd

    nc = _get_nc(BL, L)
    in_maps = _prep_host(
        input, h0, c0, emb, W_ih, W_hh, b_ih, b_hh, W_proj, b_proj, BL, L
    )
    try:
        out = run_bass_kernel_spmd(nc, in_maps, core_ids=list(range(N_CORES)))
    except Exception:
        if os.environ.pop("BASS_TRACE", None):  # retry without profiling
            out = run_bass_kernel_spmd(nc, in_maps, core_ids=list(range(N_CORES)))
        else:
            raise
    global LAST_RESULTS
    LAST_RESULTS = out
    res = out.results

    scores = np.empty((L, B, V), np.float32)
    h = np.empty((1, B, H), np.float32)
    c = np.empty((1, B, H), np.float32)
    for ci in range(N_CORES):
        sl = slice(ci * BL, (ci + 1) * BL)
        scores[:, sl, :] = res[ci]["scores"]
        h[0, sl, :] = res[ci]["houtT"].T
        c[0, sl, :] = res[ci]["coutT"].T
    return scores, (h, c)


# revision 9
# speedup vs baseline: 1.2023x; 1.0380x over previous
"""Trainium2 Bass kernel for a char-decoder LSTM step loop.

Computation (per timestep t, PyTorch LSTM gate order i,f,g,o):
    x_t   = emb[input_t]                       (B, E)
    gates = x_t @ W_ih.T + h @ W_hh.T + b      (B, 4H)
    c     = sig(f)*c + sig(i)*tanh(g)
    h     = sig(o)*tanh(c)
    s_t   = h @ W_proj.T + b_proj              (B, V)
Returns (scores(L,B,V), (h(1,B,H), c(1,B,H))).

Strategy: data-parallel over batch across 8 NeuronCores (B=16384 -> 2048/core).
On-chip layout keeps the recurrent state TRANSPOSED (H on partitions, batch on
the free dim) so the W_hh matmul needs no per-step transposes:
    gates.T (4H, B) = W_hh.T(stationary).T @ h.T(moving) + W_ih.T.T @ x.T
Gate tiles are produced as 8 chunks of (128 gate-rows, B) in PSUM, activated on
ScalarE (bias fused, sigmoid/tanh share one table set), c-state kept fp32 on
VectorE, h cast to bf16 for the next matmul.  The per-step projection runs
batch-major (lhsT = slice of transposed h state) so scores land in PSUM already
in output layout.  The embedding gather is done on the host (numpy fancy
indexing), as is the final h/c transpose; both are cheap weight/layout-only
transforms.
"""

import os
import sys

import numpy as np

sys.path.insert(0, "/opt/trn_rl_repo")

import ml_dtypes

bf16 = ml_dtypes.bfloat16

L, B, H, E, V = 32, 16384, 256, 50, 96
N_CORES = 8
BL = B // N_CORES  # per-core batch

_F32 = None
_BF16 = None


def _dt():
    import concourse.mybir as mybir

    return mybir.dt.float32, mybir.dt.bfloat16


def build_nc(bl: int, steps: int):
    """Build the Bass program for one core processing `bl` batch elements for
    `steps` timesteps. Returns the compiled Bass object."""
    import concourse.mybir as mybir
    from concourse import bacc
    import concourse.tile as tile

    f32 = mybir.dt.float32
    b16 = mybir.dt.bfloat16
    ACT = mybir.ActivationFunctionType

    assert bl % 128 == 0
    KT = H // 128  # 2 k-tiles over the hidden dim
    MT = 4 * H // 128  # 8 gate-row chunks

    nc = bacc.Bacc("TRN2", target_bir_lowering=False, debug=False)

    xT_d = nc.dram_tensor("xT", [steps, E, bl], b16, kind="ExternalInput")
    h0T_d = nc.dram_tensor("h0T", [H, bl], f32, kind="ExternalInput")
    c0T_d = nc.dram_tensor("c0T", [H, bl], f32, kind="ExternalInput")
    whhT_d = nc.dram_tensor("whhT", [H, 4 * H], b16, kind="ExternalInput")
    wihT_d = nc.dram_tensor("wihT", [E, 4 * H], b16, kind="ExternalInput")
    wprojT_d = nc.dram_tensor("wprojT", [H, V], b16, kind="ExternalInput")
    bias_d = nc.dram_tensor("bias", [128, MT], f32, kind="ExternalInput")
    bproj_d = nc.dram_tensor("bprojr", [128, V], f32, kind="ExternalInput")

    scores_d = nc.dram_tensor("scores", [steps, bl, V], f32, kind="ExternalOutput")
    houtT_d = nc.dram_tensor("houtT", [H, bl], f32, kind="ExternalOutput")
    coutT_d = nc.dram_tensor("coutT", [H, bl], f32, kind="ExternalOutput")

    # chunking of the per-core batch (free dim)
    NH = 1024 if bl % 1024 == 0 else bl  # ACT/DVE chunk
    NN = 512 if NH % 512 == 0 else NH  # matmul free-dim chunk

    with tile.TileContext(nc) as tc:
        with (
            tc.tile_pool(name="singles", bufs=1) as singles,
            tc.tile_pool(name="gates", bufs=2) as gates_pool,
            tc.tile_pool(name="sc", bufs=2) as sc_pool,
            tc.tile_pool(name="gpsum", bufs=3, space="PSUM") as gpsum,
            tc.tile_pool(name="ppsum", bufs=2, space="PSUM") as ppsum,
        ):
            # --- constants / weights ---
            whh_sb = singles.tile([128, KT, 4 * H], b16)
            nc.sync.dma_start(
                whh_sb[:], whhT_d.ap().rearrange("(kt p) m -> p kt m", p=128)
            )
            # W_ih.T twice: at partitions 0..E-1 and 64..64+E-1 so pairs of
            # x-matmuls can run concurrently in disjoint PE row groups.
            wih_sb = singles.tile([128, 4 * H], b16)
            nc.vector.memset(wih_sb[:], 0.0)
            nc.sync.dma_start(wih_sb[:E, :], wihT_d.ap())
            nc.sync.dma_start(wih_sb[64 : 64 + E, :], wihT_d.ap())
            wproj_sb = singles.tile([128, KT, V], b16)
            nc.sync.dma_start(
                wproj_sb[:], wprojT_d.ap().rearrange("(kt p) v -> p kt v", p=128)
            )
            bias_sb = singles.tile([128, MT], f32)
            nc.sync.dma_start(bias_sb[:], bias_d.ap())
            bproj_sb = singles.tile([128, V], f32)
            nc.sync.dma_start(bproj_sb[:], bproj_d.ap())

            # --- state ---
            hbf = singles.tile([128, KT, bl], b16)  # h in bf16 (matmul operand)
            cT = singles.tile([128, KT, bl], f32)  # c in fp32
            h32 = singles.tile([128, KT, bl], f32)  # fp32 h (final output only)
            nc.sync.dma_start(h32[:], h0T_d.ap().rearrange("(kt p) n -> p kt n", p=128))
            nc.vector.tensor_copy(hbf[:], h32[:])
            nc.sync.dma_start(cT[:], c0T_d.ap().rearrange("(kt p) n -> p kt n", p=128))

            # x double buffers, duplicated at partition 64 for row packing
            xts = []
            for i in range(2):
                t_ = singles.tile([128, bl], b16, tag=f"xt{i}")
                xts.append(t_)
            nc.sync.dma_start(xts[0][:E, :], xT_d.ap()[0])
            nc.sync.dma_start(xts[0][64 : 64 + E, :], xT_d.ap()[0])

            n_nh = bl // NH
            n_nn = NH // NN
            n_mb = NH // 128

            for t in range(steps):
                xt = xts[t % 2]
                if t + 1 < steps:
                    nc.sync.dma_start(xts[(t + 1) % 2][:E, :], xT_d.ap()[t + 1])
                    nc.sync.dma_start(
                        xts[(t + 1) % 2][64 : 64 + E, :], xT_d.ap()[t + 1]
                    )

                score_sb = sc_pool.tile([128, bl // 128, V], f32, tag="score")

                for nh in range(n_nh):
                    h0_, h1_ = nh * NH, (nh + 1) * NH
                    nsl = slice(h0_, h1_)

                    # per-chunk gate tiles (i,f,g fp32 feed the c update;
                    # o and tanh_c bf16 feed only the h/scores path)
                    sig_i = gates_pool.tile([128, KT, NH], f32, tag="sig_i")
                    sig_f = gates_pool.tile([128, KT, NH], f32, tag="sig_f")
                    tg = gates_pool.tile([128, KT, NH], f32, tag="tg")
                    sig_o = gates_pool.tile([128, KT, NH], b16, tag="sig_o")
                    thc = gates_pool.tile([128, KT, NH], b16, tag="thc")
                    gdst = [sig_i, sig_i, sig_f, sig_f, tg, tg, sig_o, sig_o]
                    gfun = [ACT.Sigmoid] * 4 + [ACT.Tanh] * 2 + [ACT.Sigmoid] * 2

                    for mp in range(MT // 2):
                        m0, m1 = 2 * mp, 2 * mp + 1
                        ps0 = gpsum.tile([128, NH], f32, tag="gps")
                        ps1 = gpsum.tile([128, NH], f32, tag="gps")
                        # x block first (start=True): row-tiled pairs run
                        # concurrently and never interleave with full-row MMs;
                        # also gives PE h-independent work at step boundaries
                        for nn_ in range(n_nn):
                            a = nh * NH + nn_ * NN
                            sl = slice(a, a + NN)
                            pl0 = ps0[:, nn_ * NN : (nn_ + 1) * NN]
                            pl1 = ps1[:, nn_ * NN : (nn_ + 1) * NN]
                            nc.tensor.matmul(
                                pl0,
                                wih_sb[:E, m0 * 128 : (m0 + 1) * 128],
                                xt[:E, sl],
                                start=True,
                                stop=False,
                                tile_position=(0, 0),
                            )
                            nc.tensor.matmul(
                                pl1,
                                wih_sb[64 : 64 + E, m1 * 128 : (m1 + 1) * 128],
                                xt[64 : 64 + E, sl],
                                start=True,
                                stop=False,
                                tile_position=(64, 0),
                            )
                        for nn_ in range(n_nn):
                            a = nh * NH + nn_ * NN
                            sl = slice(a, a + NN)
                            pl0 = ps0[:, nn_ * NN : (nn_ + 1) * NN]
                            pl1 = ps1[:, nn_ * NN : (nn_ + 1) * NN]
                            for k in range(KT):
                                nc.tensor.matmul(
                                    pl0,
                                    whh_sb[:, k, m0 * 128 : (m0 + 1) * 128],
                                    hbf[:, k, sl],
                                    start=False,
                                    stop=(k == KT - 1),
                                )
                                nc.tensor.matmul(
                                    pl1,
                                    whh_sb[:, k, m1 * 128 : (m1 + 1) * 128],
                                    hbf[:, k, sl],
                                    start=False,
                                    stop=(k == KT - 1),
                                )
                        nc.scalar.activation(
                            out=gdst[m0][:, m0 % 2, :],
                            in_=ps0[:],
                            func=gfun[m0],
                            bias=bias_sb[:, m0 : m0 + 1],
                        )
                        nc.scalar.activation(
                            out=gdst[m1][:, m1 % 2, :],
                            in_=ps1[:],
                            func=gfun[m1],
                            bias=bias_sb[:, m1 : m1 + 1],
                        )

                    # c = sig_f*c + sig_i*tanh(g); h = sig_o*tanh(c)
                    nc.vector.tensor_mul(tg[:], tg[:], sig_i[:])
                    nc.vector.tensor_mul(cT[:, :, nsl], cT[:, :, nsl], sig_f[:])
                    nc.vector.tensor_add(cT[:, :, nsl], cT[:, :, nsl], tg[:])
                    nc.scalar.activation(
                        out=thc[:], in_=cT[:, :, nsl], func=ACT.Tanh
                    )
                    nc.vector.tensor_mul(hbf[:, :, nsl], sig_o[:], thc[:])
                    if t == steps - 1:
                        nc.vector.tensor_mul(h32[:, :, nsl], sig_o[:], thc[:])

                    # projection, 4 batch-chunks per PSUM bank
                    for g0 in range(0, n_mb, 4):
                        gs = min(4, n_mb - g0)
                        pp = ppsum.tile([128, 4, V], f32, tag="pps")
                        for j in range(gs):
                            col = nh * NH + (g0 + j) * 128
                            nc.tensor.matmul(
                                pp[:, j, :],
                                hbf[:, 0, col : col + 128],
                                wproj_sb[:, 0, :],
                                start=True,
                                stop=False,
                            )
                            nc.tensor.matmul(
                                pp[:, j, :],
                                hbf[:, 1, col : col + 128],
                                wproj_sb[:, 1, :],
                                start=False,
                                stop=True,
                            )
                        gcol = nh * n_mb + g0
                        nc.vector.tensor_add(
                            score_sb[:, gcol : gcol + gs, :],
                            pp[:, :gs, :],
                            bproj_sb[:, None, :].to_broadcast((128, gs, V)),
                        )

                nc.sync.dma_start(
                    scores_d.ap()[t].rearrange("(mb p) v -> p mb v", p=128),
                    score_sb[:],
                )

            nc.sync.dma_start(
                houtT_d.ap().rearrange("(kt p) n -> p kt n", p=128), h32[:]
            )
            nc.sync.dma_start(
                coutT_d.ap().rearrange("(kt p) n -> p kt n", p=128), cT[:]
            )

    nc.compile()
    return nc


def _prep_host(input, h0, c0, emb, W_ih, W_hh, b_ih, b_hh, W_proj, b_proj, bl, steps):
    """Host-side data prep. Returns per-core in_maps."""
    input = np.asarray(input)
    emb = np.asarray(emb, dtype=np.float32)
    x = emb[input]  # (L, B, E) f32
    xT = np.ascontiguousarray(x.transpose(0, 2, 1)).astype(bf16)  # (L, E, B)
    h0T = np.ascontiguousarray(np.asarray(h0, np.float32)[0].T)  # (H, B)
    c0T = np.ascontiguousarray(np.asarray(c0, np.float32)[0].T)
    whhT = np.ascontiguousarray(np.asarray(W_hh, np.float32).T).astype(bf16)
    wihT = np.ascontiguousarray(np.asarray(W_ih, np.float32).T).astype(bf16)
    wprojT = np.ascontiguousarray(np.asarray(W_proj, np.float32).T).astype(bf16)
    b = (np.asarray(b_ih, np.float32) + np.asarray(b_hh, np.float32)).astype(
        np.float32
    )
    bias = np.ascontiguousarray(b.reshape(4 * H // 128, 128).T)  # (128, MT)
    bprojr = np.ascontiguousarray(
        np.tile(np.asarray(b_proj, np.float32)[None, :], (128, 1))
    )

    n_cores = x.shape[1] // bl
    in_maps = []
    for c in range(n_cores):
        sl = slice(c * bl, (c + 1) * bl)
        in_maps.append(
            {
                "xT": np.ascontiguousarray(xT[:steps, :, sl]),
                "h0T": np.ascontiguousarray(h0T[:, sl]),
                "c0T": np.ascontiguousarray(c0T[:, sl]),
                "whhT": whhT,
                "wihT": wihT,
                "wprojT": wprojT,
                "bias": bias,
                "bprojr": bprojr,
            }
        )
    return in_maps


_NC_CACHE = {}
LAST_RESULTS = None  # BassKernelResults of the most recent run (for profiling)


def _get_nc(bl, steps):
    key = (bl, steps)
    if key not in _NC_CACHE:
        _NC_CACHE[key] = build_nc(bl, steps)
    return _NC_CACHE[key]


def kernel(input, h0, c0, emb, W_ih, W_hh, b_ih, b_hh, W_proj, b_proj):
    from concourse.bass_utils import run_bass_kernel_spmd

    nc = _get_nc(BL, L)
    in_maps = _prep_host(
        input, h0, c0, emb, W_ih, W_hh, b_ih, b_hh, W_proj, b_proj, BL, L
    )
    try:
        out = run_bass_kernel_spmd(nc, in_maps, core_ids=list(range(N_CORES)))
    except Exception:
        if os.environ.pop("BASS_TRACE", None):  # retry without profiling
            out = run_bass_kernel_spmd(nc, in_maps, core_ids=list(range(N_CORES)))
        else:
            raise
    global LAST_RESULTS
    LAST_RESULTS = out
    res = out.results

    scores = np.empty((L, B, V), np.float32)
    h = np.empty((1, B, H), np.float32)
    c = np.empty((1, B, H), np.float32)
    for ci in range(N_CORES):
        sl = slice(ci * BL, (ci + 1) * BL)
        scores[:, sl, :] = res[ci]["scores"]
        h[0, sl, :] = res[ci]["houtT"].T
        c[0, sl, :] = res[ci]["coutT"].T
    return scores, (h, c)


# revision 14
# speedup vs baseline: 1.3297x; 1.1059x over previous
"""Trainium2 Bass kernel for a char-decoder LSTM step loop.

Computation (per timestep t, PyTorch LSTM gate order i,f,g,o):
    x_t   = emb[input_t]                       (B, E)
    gates = x_t @ W_ih.T + h @ W_hh.T + b      (B, 4H)
    c     = sig(f)*c + sig(i)*tanh(g)
    h     = sig(o)*tanh(c)
    s_t   = h @ W_proj.T + b_proj              (B, V)
Returns (scores(L,B,V), (h(1,B,H), c(1,B,H))).

Strategy: data-parallel over batch across 8 NeuronCores (B=16384 -> 2048/core).
On-chip layout keeps the recurrent state TRANSPOSED (H on partitions, batch on
the free dim) so the W_hh matmul needs no per-step transposes:
    gates.T (4H, B) = W_hh.T(stationary).T @ h.T(moving) + W_ih.T.T @ x.T
Gate tiles are produced as 8 chunks of (128 gate-rows, B) in PSUM, activated on
ScalarE (bias fused, sigmoid/tanh share one table set), c-state kept fp32 on
VectorE, h cast to bf16 for the next matmul.  The per-step projection runs
batch-major (lhsT = slice of transposed h state) so scores land in PSUM already
in output layout.  The embedding gather is done on the host (numpy fancy
indexing), as is the final h/c transpose; both are cheap weight/layout-only
transforms.
"""

import os
import sys

import numpy as np

sys.path.insert(0, "/opt/trn_rl_repo")

import ml_dtypes

bf16 = ml_dtypes.bfloat16

L, B, H, E, V = 32, 16384, 256, 50, 96
N_CORES = 8
BL = B // N_CORES  # per-core batch

def build_nc(bl: int, steps: int):
    """Build the Bass program for one core processing `bl` batch elements for
    `steps` timesteps. Returns the compiled Bass object."""
    import concourse.mybir as mybir
    from concourse import bacc
    import concourse.tile as tile

    f32 = mybir.dt.float32
    b16 = mybir.dt.bfloat16
    ACT = mybir.ActivationFunctionType

    assert bl % 128 == 0
    KT = H // 128  # 2 k-tiles over the hidden dim
    MT = 4 * H // 128  # 8 gate-row chunks

    nc = bacc.Bacc("TRN2", target_bir_lowering=False, debug=False)

    xT_d = nc.dram_tensor("xT", [steps, E, bl], b16, kind="ExternalInput")
    h0T_d = nc.dram_tensor("h0T", [H, bl], f32, kind="ExternalInput")
    c0T_d = nc.dram_tensor("c0T", [H, bl], f32, kind="ExternalInput")
    whhT_d = nc.dram_tensor("whhT", [H, 4 * H], b16, kind="ExternalInput")
    wihT_d = nc.dram_tensor("wihT", [E, 4 * H], b16, kind="ExternalInput")
    wprojT_d = nc.dram_tensor("wprojT", [H, V], b16, kind="ExternalInput")
    bias_d = nc.dram_tensor("bias", [128, MT], f32, kind="ExternalInput")
    bproj_d = nc.dram_tensor("bprojr", [128, V], f32, kind="ExternalInput")

    scores_d = nc.dram_tensor("scores", [steps, bl, V], f32, kind="ExternalOutput")
    houtT_d = nc.dram_tensor("houtT", [H, bl], f32, kind="ExternalOutput")
    coutT_d = nc.dram_tensor("coutT", [H, bl], f32, kind="ExternalOutput")

    # chunking of the per-core batch (free dim)
    NH = 1024 if bl % 1024 == 0 else bl  # ACT/DVE chunk
    NN = 512 if NH % 512 == 0 else NH  # matmul free-dim chunk

    with tile.TileContext(nc) as tc:
        with (
            tc.tile_pool(name="singles", bufs=1) as singles,
            tc.tile_pool(name="gates", bufs=2) as gates_pool,
            tc.tile_pool(name="sc", bufs=2) as sc_pool,
            tc.tile_pool(name="gpsum", bufs=3, space="PSUM") as gpsum,
            tc.tile_pool(name="ppsum", bufs=2, space="PSUM") as ppsum,
        ):
            # --- constants / weights ---
            whh_sb = singles.tile([128, KT, 4 * H], b16)
            nc.sync.dma_start(
                whh_sb[:], whhT_d.ap().rearrange("(kt p) m -> p kt m", p=128)
            )
            # W_ih.T zero-padded to K=128 so the x matmul shares the full-row
            # pipeline with the W_hh matmuls
            wih_sb = singles.tile([128, 4 * H], b16)
            nc.vector.memset(wih_sb[:], 0.0)
            nc.sync.dma_start(wih_sb[:E, :], wihT_d.ap())
            wproj_sb = singles.tile([128, KT, V], b16)
            nc.sync.dma_start(
                wproj_sb[:], wprojT_d.ap().rearrange("(kt p) v -> p kt v", p=128)
            )
            bias_sb = singles.tile([128, MT], f32)
            nc.sync.dma_start(bias_sb[:], bias_d.ap())
            bproj_sb = singles.tile([128, V], f32)
            nc.sync.dma_start(bproj_sb[:], bproj_d.ap())

            # --- state ---
            hbf = singles.tile([128, KT, bl], b16)  # h in bf16 (matmul operand)
            cT = singles.tile([128, KT, bl], f32)  # c in fp32
            h32 = singles.tile([128, KT, bl], f32)  # fp32 h (final output only)
            nc.sync.dma_start(h32[:], h0T_d.ap().rearrange("(kt p) n -> p kt n", p=128))
            nc.vector.tensor_copy(hbf[:], h32[:])
            nc.sync.dma_start(cT[:], c0T_d.ap().rearrange("(kt p) n -> p kt n", p=128))

            # x double buffers (zero rows E..127 once; DMA writes rows :E)
            xts = []
            for i in range(2):
                t_ = singles.tile([128, bl], b16, tag=f"xt{i}")
                nc.vector.memset(t_[:], 0.0)
                xts.append(t_)
            nc.sync.dma_start(xts[0][:E, :], xT_d.ap()[0])

            n_nh = bl // NH
            n_nn = NH // NN
            n_mb = NH // 128

            for t in range(steps):
                xt = xts[t % 2]
                if t + 1 < steps:
                    nc.sync.dma_start(xts[(t + 1) % 2][:E, :], xT_d.ap()[t + 1])

                score_sb = sc_pool.tile([128, bl // 128, V], f32, tag="score")

                for nh in range(n_nh):
                    h0_, h1_ = nh * NH, (nh + 1) * NH
                    nsl = slice(h0_, h1_)

                    # per-chunk gate tiles (i,f,g fp32 feed the c update;
                    # o and tanh_c bf16 feed only the h/scores path)
                    sig_i = gates_pool.tile([128, KT, NH], f32, tag="sig_i")
                    sig_f = gates_pool.tile([128, KT, NH], f32, tag="sig_f")
                    tg = gates_pool.tile([128, KT, NH], f32, tag="tg")
                    sig_o = gates_pool.tile([128, KT, NH], b16, tag="sig_o")
                    thc = gates_pool.tile([128, KT, NH], b16, tag="thc")
                    gdst = [sig_i, sig_i, sig_f, sig_f, tg, tg, sig_o, sig_o]
                    gfun = [ACT.Sigmoid] * 4 + [ACT.Tanh] * 2 + [ACT.Sigmoid] * 2

                    for m in range(MT):
                        ps = gpsum.tile([128, NH], f32, tag="gps")
                        for nn_ in range(n_nn):
                            a = nh * NH + nn_ * NN
                            sl = slice(a, a + NN)
                            pl = ps[:, nn_ * NN : (nn_ + 1) * NN]
                            nc.tensor.matmul(
                                pl,
                                whh_sb[:, 0, m * 128 : (m + 1) * 128],
                                hbf[:, 0, sl],
                                start=True,
                                stop=False,
                            )
                            nc.tensor.matmul(
                                pl,
                                whh_sb[:, 1, m * 128 : (m + 1) * 128],
                                hbf[:, 1, sl],
                                start=False,
                                stop=False,
                            )
                            nc.tensor.matmul(
                                pl,
                                wih_sb[:, m * 128 : (m + 1) * 128],
                                xt[:, sl],
                                start=False,
                                stop=True,
                            )
                        nc.scalar.activation(
                            out=gdst[m][:, m % 2, :],
                            in_=ps[:],
                            func=gfun[m],
                            bias=bias_sb[:, m : m + 1],
                        )

                    # c = sig_f*c + sig_i*tanh(g); h = sig_o*tanh(c)
                    nc.vector.tensor_mul(tg[:], tg[:], sig_i[:])
                    nc.vector.tensor_mul(cT[:, :, nsl], cT[:, :, nsl], sig_f[:])
                    nc.vector.tensor_add(cT[:, :, nsl], cT[:, :, nsl], tg[:])
                    nc.scalar.activation(
                        out=thc[:], in_=cT[:, :, nsl], func=ACT.Tanh
                    )
                    nc.vector.tensor_mul(hbf[:, :, nsl], sig_o[:], thc[:])
                    if t == steps - 1:
                        nc.vector.tensor_mul(h32[:, :, nsl], sig_o[:], thc[:])

                    # projection, 4 batch-chunks per PSUM bank
                    for g0 in range(0, n_mb, 4):
                        gs = min(4, n_mb - g0)
                        pp = ppsum.tile([128, 4, V], f32, tag="pps")
                        for j in range(gs):
                            col = nh * NH + (g0 + j) * 128
                            nc.tensor.matmul(
                                pp[:, j, :],
                                hbf[:, 0, col : col + 128],
                                wproj_sb[:, 0, :],
                                start=True,
                                stop=False,
                            )
                            nc.tensor.matmul(
                                pp[:, j, :],
                                hbf[:, 1, col : col + 128],
                                wproj_sb[:, 1, :],
                                start=False,
                                stop=True,
                            )
                        gcol = nh * n_mb + g0
                        nc.vector.tensor_add(
                            score_sb[:, gcol : gcol + gs, :],
                            pp[:, :gs, :],
                            bproj_sb[:, None, :].to_broadcast((128, gs, V)),
                        )

                nc.sync.dma_start(
                    scores_d.ap()[t].rearrange("(mb p) v -> p mb v", p=128),
                    score_sb[:],
                )

            nc.sync.dma_start(
                houtT_d.ap().rearrange("(kt p) n -> p kt n", p=128), h32[:]
            )
            nc.sync.dma_start(
                coutT_d.ap().rearrange("(kt p) n -> p kt n", p=128), cT[:]
            )

    nc.compile()
    return nc


def _prep_host(input, h0, c0, emb, W_ih, W_hh, b_ih, b_hh, W_proj, b_proj, bl, steps):
    """Host-side data prep. Returns per-core in_maps."""
    input = np.asarray(input)
    emb = np.asarray(emb, dtype=np.float32)
    x = emb[input]  # (L, B, E) f32
    xT = np.ascontiguousarray(x.transpose(0, 2, 1)).astype(bf16)  # (L, E, B)
    h0T = np.ascontiguousarray(np.asarray(h0, np.float32)[0].T)  # (H, B)
    c0T = np.ascontiguousarray(np.asarray(c0, np.float32)[0].T)
    whhT = np.ascontiguousarray(np.asarray(W_hh, np.float32).T).astype(bf16)
    wihT = np.ascontiguousarray(np.asarray(W_ih, np.float32).T).astype(bf16)
    wprojT = np.ascontiguousarray(np.asarray(W_proj, np.float32).T).astype(bf16)
    b = (np.asarray(b_ih, np.float32) + np.asarray(b_hh, np.float32)).astype(
        np.float32
    )
    bias = np.ascontiguousarray(b.reshape(4 * H // 128, 128).T)  # (128, MT)
    bprojr = np.ascontiguousarray(
        np.tile(np.asarray(b_proj, np.float32)[None, :], (128, 1))
    )

    n_cores = x.shape[1] // bl
    in_maps = []
    for c in range(n_cores):
        sl = slice(c * bl, (c + 1) * bl)
        in_maps.append(
            {
                "xT": np.ascontiguousarray(xT[:steps, :, sl]),
                "h0T": np.ascontiguousarray(h0T[:, sl]),
                "c0T": np.ascontiguousarray(c0T[:, sl]),
                "whhT": whhT,
                "wihT": wihT,
                "wprojT": wprojT,
                "bias": bias,
                "bprojr": bprojr,
            }
        )
    return in_maps


_NC_CACHE = {}
LAST_RESULTS = None  # BassKernelResults of the most recent run (for profiling)


def _get_nc(bl, steps):
    key = (bl, steps)
    if key not in _NC_CACHE:
        _NC_CACHE[key] = build_nc(bl, steps)
    return _NC_CACHE[key]


def kernel(input, h0, c0, emb, W_ih, W_hh, b_ih, b_hh, W_proj, b_proj):
    from concourse.bass_utils import run_bass_kernel_spmd

    nc = _get_nc(BL, L)
    in_maps = _prep_host(
        input, h0, c0, emb, W_ih, W_hh, b_ih, b_hh, W_proj, b_proj, BL, L
    )
    try:
        out = run_bass_kernel_spmd(nc, in_maps, core_ids=list(range(N_CORES)))
    except Exception:
        if os.environ.pop("BASS_TRACE", None):  # retry without profiling
            out = run_bass_kernel_spmd(nc, in_maps, core_ids=list(range(N_CORES)))
        else:
            raise
    global LAST_RESULTS
    LAST_RESULTS = out
    res = out.results

    scores = np.empty((L, B, V), np.float32)
    h = np.empty((1, B, H), np.float32)
    c = np.empty((1, B, H), np.float32)
    for ci in range(N_CORES):
        sl = slice(ci * BL, (ci + 1) * BL)
        scores[:, sl, :] = res[ci]["scores"]
        h[0, sl, :] = res[ci]["houtT"].T
        c[0, sl, :] = res[ci]["coutT"].T
    return scores, (h, c)


# revision 15
# speedup vs baseline: 1.3748x; 1.0339x over previous
"""Trainium2 Bass kernel for a char-decoder LSTM step loop.

Computation (per timestep t, PyTorch LSTM gate order i,f,g,o):
    x_t   = emb[input_t]                       (B, E)
    gates = x_t @ W_ih.T + h @ W_hh.T + b      (B, 4H)
    c     = sig(f)*c + sig(i)*tanh(g)
    h     = sig(o)*tanh(c)
    s_t   = h @ W_proj.T + b_proj              (B, V)
Returns (scores(L,B,V), (h(1,B,H), c(1,B,H))).

Strategy: data-parallel over batch across 8 NeuronCores (B=16384 -> 2048/core).
On-chip layout keeps the recurrent state TRANSPOSED (H on partitions, batch on
the free dim) so the W_hh matmul needs no per-step transposes:
    gates.T (4H, B) = W_hh.T(stationary).T @ h.T(moving) + W_ih.T.T @ x.T
Gate tiles are produced as 8 chunks of (128 gate-rows, B) in PSUM, activated on
ScalarE (bias fused, sigmoid/tanh share one table set), c-state kept fp32 on
VectorE, h cast to bf16 for the next matmul.  The per-step projection runs
batch-major (lhsT = slice of transposed h state) so scores land in PSUM already
in output layout.  The embedding gather is done on the host (numpy fancy
indexing), as is the final h/c transpose; both are cheap weight/layout-only
transforms.
"""

import os
import sys

import numpy as np

sys.path.insert(0, "/opt/trn_rl_repo")

import ml_dtypes

bf16 = ml_dtypes.bfloat16

L, B, H, E, V = 32, 16384, 256, 50, 96
N_CORES = 8
BL = B // N_CORES  # per-core batch

def build_nc(bl: int, steps: int):
    """Build the Bass program for one core processing `bl` batch elements for
    `steps` timesteps. Returns the compiled Bass object."""
    import concourse.mybir as mybir
    from concourse import bacc
    import concourse.tile as tile

    f32 = mybir.dt.float32
    b16 = mybir.dt.bfloat16
    ACT = mybir.ActivationFunctionType

    assert bl % 128 == 0
    KT = H // 128  # 2 k-tiles over the hidden dim
    MT = 4 * H // 128  # 8 gate-row chunks

    nc = bacc.Bacc("TRN2", target_bir_lowering=False, debug=False)

    xT_d = nc.dram_tensor("xT", [steps, E, bl], b16, kind="ExternalInput")
    h0T_d = nc.dram_tensor("h0T", [H, bl], f32, kind="ExternalInput")
    c0T_d = nc.dram_tensor("c0T", [H, bl], f32, kind="ExternalInput")
    whhT_d = nc.dram_tensor("whhT", [H, 4 * H], b16, kind="ExternalInput")
    wihT_d = nc.dram_tensor("wihT", [E, 4 * H], b16, kind="ExternalInput")
    wprojT_d = nc.dram_tensor("wprojT", [H, V], b16, kind="ExternalInput")
    bias_d = nc.dram_tensor("bias", [128, MT], f32, kind="ExternalInput")
    bproj_d = nc.dram_tensor("bprojr", [128, V], f32, kind="ExternalInput")

    scores_d = nc.dram_tensor("scores", [steps, bl, V], f32, kind="ExternalOutput")
    houtT_d = nc.dram_tensor("houtT", [H, bl], f32, kind="ExternalOutput")
    coutT_d = nc.dram_tensor("coutT", [H, bl], f32, kind="ExternalOutput")

    # chunking of the per-core batch (free dim)
    NH = 1024 if bl % 1024 == 0 else bl  # ACT/DVE chunk
    NN = 512 if NH % 512 == 0 else NH  # matmul free-dim chunk

    with tile.TileContext(nc) as tc:
        with (
            tc.tile_pool(name="singles", bufs=1) as singles,
            tc.tile_pool(name="gates", bufs=2) as gates_pool,
            tc.tile_pool(name="sc", bufs=2) as sc_pool,
            tc.tile_pool(name="gpsum", bufs=3, space="PSUM") as gpsum,
            tc.tile_pool(name="ppsum", bufs=2, space="PSUM") as ppsum,
        ):
            # --- constants / weights ---
            whh_sb = singles.tile([128, KT, 4 * H], b16)
            nc.sync.dma_start(
                whh_sb[:], whhT_d.ap().rearrange("(kt p) m -> p kt m", p=128)
            )
            # W_ih.T zero-padded to K=128 so the x matmul shares the full-row
            # pipeline with the W_hh matmuls
            wih_sb = singles.tile([128, 4 * H], b16)
            nc.vector.memset(wih_sb[:], 0.0)
            nc.sync.dma_start(wih_sb[:E, :], wihT_d.ap())
            wproj_sb = singles.tile([128, KT, V], b16)
            nc.sync.dma_start(
                wproj_sb[:], wprojT_d.ap().rearrange("(kt p) v -> p kt v", p=128)
            )
            bias_sb = singles.tile([128, MT], f32)
            nc.sync.dma_start(bias_sb[:], bias_d.ap())
            bproj_sb = singles.tile([128, V], f32)
            nc.sync.dma_start(bproj_sb[:], bproj_d.ap())

            # --- state ---
            hbf = singles.tile([128, KT, bl], b16)  # h in bf16 (matmul operand)
            cT = singles.tile([128, KT, bl], f32)  # c in fp32
            h32 = singles.tile([128, KT, bl], f32)  # fp32 h (final output only)
            nc.sync.dma_start(h32[:], h0T_d.ap().rearrange("(kt p) n -> p kt n", p=128))
            nc.vector.tensor_copy(hbf[:], h32[:])
            nc.sync.dma_start(cT[:], c0T_d.ap().rearrange("(kt p) n -> p kt n", p=128))

            # x double buffers (zero rows E..127 once; DMA writes rows :E)
            xts = []
            for i in range(2):
                t_ = singles.tile([128, bl], b16, tag=f"xt{i}")
                nc.vector.memset(t_[:], 0.0)
                xts.append(t_)
            nc.sync.dma_start(xts[0][:E, :], xT_d.ap()[0])

            n_nh = bl // NH
            n_nn = NH // NN
            n_mb = NH // 128

            for t in range(steps):
                xt = xts[t % 2]
                if t + 1 < steps:
                    nc.sync.dma_start(xts[(t + 1) % 2][:E, :], xT_d.ap()[t + 1])

                score_sb = sc_pool.tile([128, bl // 128, V], f32, tag="score")

                for nh in range(n_nh):
                    h0_, h1_ = nh * NH, (nh + 1) * NH
                    nsl = slice(h0_, h1_)

                    # per-chunk gate tiles (i,f,g fp32 feed the c update;
                    # o and tanh_c bf16 feed only the h/scores path)
                    sig_i = gates_pool.tile([128, KT, NH], f32, tag="sig_i")
                    sig_f = gates_pool.tile([128, KT, NH], f32, tag="sig_f")
                    tg = gates_pool.tile([128, KT, NH], f32, tag="tg")
                    sig_o = gates_pool.tile([128, KT, NH], b16, tag="sig_o")
                    thc = gates_pool.tile([128, KT, NH], b16, tag="thc")
                    gdst = [sig_i, sig_i, sig_f, sig_f, tg, tg, sig_o, sig_o]
                    gfun = [ACT.Sigmoid] * 4 + [ACT.Tanh] * 2 + [ACT.Sigmoid] * 2

                    for m in range(MT):
                        ps = gpsum.tile([128, NH], f32, tag="gps")
                        for nn_ in range(n_nn):
                            a = nh * NH + nn_ * NN
                            sl = slice(a, a + NN)
                            pl = ps[:, nn_ * NN : (nn_ + 1) * NN]
                            nc.tensor.matmul(
                                pl,
                                whh_sb[:, 0, m * 128 : (m + 1) * 128],
                                hbf[:, 0, sl],
                                start=True,
                                stop=False,
                            )
                            nc.tensor.matmul(
                                pl,
                                whh_sb[:, 1, m * 128 : (m + 1) * 128],
                                hbf[:, 1, sl],
                                start=False,
                                stop=False,
                            )
                            nc.tensor.matmul(
                                pl,
                                wih_sb[:, m * 128 : (m + 1) * 128],
                                xt[:, sl],
                                start=False,
                                stop=True,
                            )
                        nc.scalar.activation(
                            out=gdst[m][:, m % 2, :],
                            in_=ps[:],
                            func=gfun[m],
                            bias=bias_sb[:, m : m + 1],
                        )

                    # c = sig_f*c + sig_i*tanh(g); h = sig_o*tanh(c)
                    # split into NN-wide quarters: shortens the critical path
                    # from last gate ACT to hbf availability for step t+1
                    for q in range(NH // NN):
                        qt = (slice(None), slice(None), slice(q * NN, (q + 1) * NN))
                        qc = (
                            slice(None),
                            slice(None),
                            slice(h0_ + q * NN, h0_ + (q + 1) * NN),
                        )
                        nc.vector.tensor_mul(tg[qt], tg[qt], sig_i[qt])
                        nc.vector.tensor_mul(cT[qc], cT[qc], sig_f[qt])
                        nc.vector.tensor_add(cT[qc], cT[qc], tg[qt])
                        nc.scalar.activation(
                            out=thc[qt], in_=cT[qc], func=ACT.Tanh
                        )
                        nc.vector.tensor_mul(hbf[qc], sig_o[qt], thc[qt])
                        if t == steps - 1:
                            nc.vector.tensor_mul(h32[qc], sig_o[qt], thc[qt])

                    # projection, 4 batch-chunks per PSUM bank
                    for g0 in range(0, n_mb, 4):
                        gs = min(4, n_mb - g0)
                        pp = ppsum.tile([128, 4, V], f32, tag="pps")
                        for j in range(gs):
                            col = nh * NH + (g0 + j) * 128
                            nc.tensor.matmul(
                                pp[:, j, :],
                                hbf[:, 0, col : col + 128],
                                wproj_sb[:, 0, :],
                                start=True,
                                stop=False,
                            )
                            nc.tensor.matmul(
                                pp[:, j, :],
                                hbf[:, 1, col : col + 128],
                                wproj_sb[:, 1, :],
                                start=False,
                                stop=True,
                            )
                        gcol = nh * n_mb + g0
                        nc.vector.tensor_add(
                            score_sb[:, gcol : gcol + gs, :],
                            pp[:, :gs, :],
                            bproj_sb[:, None, :].to_broadcast((128, gs, V)),
                        )

                nc.sync.dma_start(
                    scores_d.ap()[t].rearrange("(mb p) v -> p mb v", p=128),
                    score_sb[:],
                )

            nc.sync.dma_start(
                houtT_d.ap().rearrange("(kt p) n -> p kt n", p=128), h32[:]
            )
            nc.sync.dma_start(
                coutT_d.ap().rearrange("(kt p) n -> p kt n", p=128), cT[:]
            )

    nc.compile()
    return nc


def _prep_host(input, h0, c0, emb, W_ih, W_hh, b_ih, b_hh, W_proj, b_proj, bl, steps):
    """Host-side data prep. Returns per-core in_maps."""
    input = np.asarray(input)
    emb = np.asarray(emb, dtype=np.float32)
    x = emb[input]  # (L, B, E) f32
    xT = np.ascontiguousarray(x.transpose(0, 2, 1)).astype(bf16)  # (L, E, B)
    h0T = np.ascontiguousarray(np.asarray(h0, np.float32)[0].T)  # (H, B)
    c0T = np.ascontiguousarray(np.asarray(c0, np.float32)[0].T)
    whhT = np.ascontiguousarray(np.asarray(W_hh, np.float32).T).astype(bf16)
    wihT = np.ascontiguousarray(np.asarray(W_ih, np.float32).T).astype(bf16)
    wprojT = np.ascontiguousarray(np.asarray(W_proj, np.float32).T).astype(bf16)
    b = (np.asarray(b_ih, np.float32) + np.asarray(b_hh, np.float32)).astype(
        np.float32
    )
    bias = np.ascontiguousarray(b.reshape(4 * H // 128, 128).T)  # (128, MT)
    bprojr = np.ascontiguousarray(
        np.tile(np.asarray(b_proj, np.float32)[None, :], (128, 1))
    )

    n_cores = x.shape[1] // bl
    in_maps = []
    for c in range(n_cores):
        sl = slice(c * bl, (c + 1) * bl)
        in_maps.append(
            {
                "xT": np.ascontiguousarray(xT[:steps, :, sl]),
                "h0T": np.ascontiguousarray(h0T[:, sl]),
                "c0T": np.ascontiguousarray(c0T[:, sl]),
                "whhT": whhT,
                "wihT": wihT,
                "wprojT": wprojT,
                "bias": bias,
                "bprojr": bprojr,
            }
        )
    return in_maps


_NC_CACHE = {}
LAST_RESULTS = None  # BassKernelResults of the most recent run (for profiling)


def _get_nc(bl, steps):
    key = (bl, steps)
    if key not in _NC_CACHE:
        _NC_CACHE[key] = build_nc(bl, steps)
    return _NC_CACHE[key]


def kernel(input, h0, c0, emb, W_ih, W_hh, b_ih, b_hh, W_proj, b_proj):
    from concourse.bass_utils import run_bass_kernel_spmd

    nc = _get_nc(BL, L)
    in_maps = _prep_host(
        input, h0, c0, emb, W_ih, W_hh, b_ih, b_hh, W_proj, b_proj, BL, L
    )
    try:
        out = run_bass_kernel_spmd(nc, in_maps, core_ids=list(range(N_CORES)))
    except Exception:
        if os.environ.pop("BASS_TRACE", None):  # retry without profiling
            out = run_bass_kernel_spmd(nc, in_maps, core_ids=list(range(N_CORES)))
        else:
            raise
    global LAST_RESULTS
    LAST_RESULTS = out
    res = out.results

    scores = np.empty((L, B, V), np.float32)
    h = np.empty((1, B, H), np.float32)
    c = np.empty((1, B, H), np.float32)
    for ci in range(N_CORES):
        sl = slice(ci * BL, (ci + 1) * BL)
        scores[:, sl, :] = res[ci]["scores"]
        h[0, sl, :] = res[ci]["houtT"].T
        c[0, sl, :] = res[ci]["coutT"].T
    return scores, (h, c)
